# revision 1
# baseline (speedup 1.0000x reference)
"""Distributed Trainium2 Bass kernel for nn_NodeFeat (2-hop Chebyshev-style GNN
feature expansion + edge gather), 8 NeuronCores.

Node sharding per the problem's sharding hint:
  - 50000 nodes padded to 50176 = 8 x 6272; core c owns rows [6272c, 6272c+6272).
  - adjacency rows are pre-sorted; each core handles the edges whose ROW is in
    its shard, packed per 128-row tile into NCHUNK=18 chunks of 128 slots
    (dummy slots use an out-of-bounds index -> DMA descriptor skipped).
  - hop1: indirect-DMA gather of x[col] rows, scaled per-edge by
    {1, rsqrt(deg_col), sqrt(deg_col)} into a [128,192] fp16 moving operand;
    segment-sum on TensorE via a one-hot selector (is_equal of rowloc vs iota)
    accumulated in PSUM; ScalarE evacuates with the 1/deg row scale.
  - one on-chip AllGather of the per-core y1 shard between hops.
  - hop2: same machinery gathering y1 rows, then minus xs0.
  - final: edge endpoints partitioned by owner core (host all-to-all
    bookkeeping); each core gathers its [xs0|y1|xs2] rows, transposes [9,64]
    -> [64,9] on-chip, writes packed rows; host scatters into [2,32768,64,9].

All floating-point math runs on device; the host only shards, pads, reorders
and reassembles (index bookkeeping).
"""
import numpy as np

import concourse.bass as bass
import concourse.mybir as mybir
import concourse.tile as tile
from concourse.bass_utils import run_bass_kernel_spmd

# ---------------- hardcoded problem geometry ----------------
N = 50000
D = 64
EQ = 32768
P = 128
NC = 8                   # cores
NT = 49                  # row tiles per core
NSH = NT * P             # 6272 rows per core
NPAD = NSH * NC          # 50176
NCHUNK = 18              # 128-edge chunks per row tile
FCH = 66                 # final-gather chunks per core (66*128 = 8448 slots)
PC = 6                   # final-gather chunks per piece (11 pieces)
BIG = 10 ** 7            # out-of-bounds index -> DMA descriptor skipped
F32 = mybir.dt.float32
F16 = mybir.dt.float16
I32 = mybir.dt.int32
EDGE_COLS = NT * NCHUNK  # 882

_prog_cache = {}


class _TC(tile.TileContext):
    """TileContext whose final drain splits sem waits one-per-instruction
    (this walrus rejects >1 sync wait on an instruction)."""

    def _drain_and_barrier(self, tick_clock, wait_clock):
        nc = self.nc
        probe = nc.sync.nop()
        wait_clock.add_sem_waits(
            probe.ins, tile.ScopedClock({None: tick_clock.global_clock}))
        si = probe.ins.sync_info
        waits = list(si.on_wait) if si and si.on_wait else []
        if si is not None:
            si.on_wait = waits[:1]
        for w in waits[1:]:
            n2 = nc.sync.nop()
            if n2.ins.sync_info is None:
                n2.ins.sync_info = mybir.SyncInfo(on_wait=[w], on_update=[])
            else:
                n2.ins.sync_info.on_wait = [w]
        nc.sync.drain()
        nc.all_engine_barrier()
        popped = nc._tile_sem_poison_stack.pop()
        assert popped is self._sem_poison
        nc.clear_and_free_semaphores(list(self.sems.allocated().values()))
        nc.all_engine_barrier()


def _split_multi_waits(nc):
    for fn in nc.m.functions:
        for blk in fn.blocks:
            new_list = []
            for inst in blk.instructions:
                si = inst.sync_info
                waits = list(si.on_wait) if si and si.on_wait else []
                if len(waits) > 1:
                    for j, w in enumerate(waits[:-1]):
                        nop = mybir.InstNoOp(
                            name=f"{inst.name}-ws{j}",
                            engine=inst.engine,
                            ins=[], outs=[],
                            sync_info=mybir.SyncInfo(on_wait=[w], on_update=[]),
                        )
                        nc.register_instruction(nop, overwrite=True)
                        new_list.append(nop)
                    si.on_wait = waits[-1:]
                new_list.append(inst)
            blk.instructions[:] = new_list


def _dims(ap, dims):
    """Same tensor+offset as `ap`, explicit [stride(elem), nelem] dims."""
    return bass.AP(ap.tensor, ap.offset, dims)


def _build_program(ablate=()):
    """ablate: subset of {"hop1","gather1","ag","hop2","gather2","final","gatherf"}
    to SKIP (for performance ablation only — results become wrong)."""
    ab = set(ablate)
    nc = bass.Bass("TRN2", target_bir_lowering=False, debug=False, num_devices=NC)

    x_full = nc.dram_tensor("x_full", [NPAD, D], F32, kind="ExternalInput")
    x_sh = nc.dram_tensor("x_sh", [NSH, D], F32, kind="ExternalInput")
    degsh_in = nc.dram_tensor("degsh", [P, NT], F32, kind="ExternalInput")
    idx1_in = nc.dram_tensor("idx1", [P, EDGE_COLS], I32, kind="ExternalInput")
    rowloc_in = nc.dram_tensor("rowloc", [P, EDGE_COLS], F16, kind="ExternalInput")
    degcol_in = nc.dram_tensor("degcol", [P, EDGE_COLS], F32, kind="ExternalInput")
    fidx_loc_in = nc.dram_tensor("fidx_loc", [P, FCH], I32, kind="ExternalInput")
    fidx_mid_in = nc.dram_tensor("fidx_mid", [P, FCH], I32, kind="ExternalInput")
    iota_in = nc.dram_tensor("iota", [P, P], F16, kind="ExternalInput")

    out_f = nc.dram_tensor("out_f", [FCH * P, 576], F32, kind="ExternalOutput")

    y1_bounce = nc.dram_tensor("y1_bounce", [NSH, 192], F32)
    y1full = nc.dram_tensor("y1full", [NPAD, 192], F32, addr_space="Shared")
    xs0_l = nc.dram_tensor("xs0_l", [NSH, 192], F32)
    xs2_l = nc.dram_tensor("xs2_l", [NSH, 192], F32)

    eq = mybir.AluOpType.is_equal
    mult = mybir.AluOpType.mult
    sub = mybir.AluOpType.subtract
    COPY = mybir.ActivationFunctionType.Copy
    SQRT = mybir.ActivationFunctionType.Sqrt

    with _TC(nc) as tc, nc.allow_low_precision(reason="fp16 matmul operands; PSUM accumulates in f32"), \
            nc.gpsimd.register("bnd_pad") as bnd_pad, \
            nc.gpsimd.register("bnd_sh") as bnd_sh:
        nc.gpsimd.reg_mov(bnd_pad, NPAD - 1)
        nc.gpsimd.reg_mov(bnd_sh, NSH - 1)
        with (
            tc.tile_pool(name="const", bufs=1) as cp,
            tc.tile_pool(name="v1", bufs=3) as v1p,
            tc.tile_pool(name="s", bufs=3) as sp_,
            tc.tile_pool(name="v3", bufs=3) as v3p,
            tc.tile_pool(name="rq", bufs=3) as rqp,
            tc.tile_pool(name="ev", bufs=3) as evp,
            tc.tile_pool(name="x0", bufs=3) as x0p,
            tc.tile_pool(name="v2", bufs=3) as v2p,
            tc.tile_pool(name="g", bufs=2) as gp,
            tc.tile_pool(name="st", bufs=2) as stp,
            tc.tile_pool(name="psum", bufs=4, space="PSUM") as pp,
        ):
            iota_t = cp.tile([P, P], F16)
            nc.sync.dma_start(out=iota_t[:], in_=iota_in[:])
            idx1_t = cp.tile([P, EDGE_COLS], I32)
            nc.sync.dma_start(out=idx1_t[:], in_=idx1_in[:])
            rowloc_t = cp.tile([P, EDGE_COLS], F16)
            nc.sync.dma_start(out=rowloc_t[:], in_=rowloc_in[:])
            degcol_t = cp.tile([P, EDGE_COLS], F32)
            nc.sync.dma_start(out=degcol_t[:], in_=degcol_in[:])
            degsh_t = cp.tile([P, NT], F32)
            nc.sync.dma_start(out=degsh_t[:], in_=degsh_in[:])
            fidx_loc_t = cp.tile([P, FCH], I32)
            nc.sync.dma_start(out=fidx_loc_t[:], in_=fidx_loc_in[:])
            fidx_mid_t = cp.tile([P, FCH], I32)
            nc.sync.dma_start(out=fidx_mid_t[:], in_=fidx_mid_in[:])

            def build_s(t):
                s_t = sp_.tile([P, NCHUNK, P], F16, tag="s")
                rl = rowloc_t[:, t * NCHUNK:(t + 1) * NCHUNK]
                rl_b = rl.to_broadcast([P, NCHUNK, P])
                io = iota_t[:]
                io_b = _dims(io, [io.ap[0], [0, NCHUNK], io.ap[1]])
                nc.vector.tensor_tensor(out=s_t[:], in0=rl_b, in1=io_b, op=eq)
                return s_t

            # whole-shard precomputes (hoisted out of the tile loops)
            # rq_all[:, 0, :] = rsqrt(deg_col) f16, rq_all[:, 1, :] = sqrt f16
            rq_all = cp.tile([P, 2, EDGE_COLS], F16)
            q32_all = cp.tile([P, EDGE_COLS], F32)
            nc.scalar.activation(q32_all[:], degcol_t[:], SQRT)
            nc.vector.tensor_copy(out=rq_all[:, 1, :], in_=q32_all[:])
            nc.vector.reciprocal(rq_all[:, 0, :], q32_all[:])
            # degrev_all [P, NT] f32; rq0_all [P, 2, NT] f32 (row scales)
            degrev_all = cp.tile([P, NT], F32)
            nc.vector.reciprocal(degrev_all[:], degsh_t[:])
            rq0_all = cp.tile([P, 2, NT], F32)
            nc.scalar.activation(rq0_all[:, 1, :], degsh_t[:], SQRT)
            nc.vector.reciprocal(rq0_all[:, 0, :], rq0_all[:, 1, :])
            # xs0 block 0 = x (DRAM->DRAM strided copy, once)
            x0dst = _dims(xs0_l[:, 0:D], [[192, NSH], [1, D]])
            nc.sync.dma_start(out=x0dst, in_=x_sh[:])

            # ================= hop 1 =================
            for t in range(NT if "hop1" not in ab else 0):
                v_t = v1p.tile([P, NCHUNK, D], F32, tag="v1")
                if t < 3 or "gather1" in ab:
                    nc.gpsimd.memset(v_t[:], 0.0)
                for j in range(NCHUNK if "gather1" not in ab else 0):
                    col = t * NCHUNK + j
                    nc.gpsimd.indirect_dma_start(
                        out=v_t[:, j, :], out_offset=None, in_=x_full[:],
                        in_offset=bass.IndirectOffsetOnAxis(
                            ap=idx1_t[:, col:col + 1], axis=0),
                        bounds_check=bnd_pad, oob_is_err=False,
                    )
                s_t = build_s(t)
                rq = rq_all[:, :, t * NCHUNK:(t + 1) * NCHUNK]
                # v3 [P, NCHUNK, 192] fp16 = [x | x*r | x*q] per chunk
                v3 = v3p.tile([P, NCHUNK, 192], F16, tag="v3")
                b0 = v3[:, :, 0:D]
                nc.scalar.activation(b0, v_t[:], COPY)
                b12 = _dims(v3[:, :, D:3 * D],
                            [v3[:].ap[0], [192, NCHUNK], [D, 2], [1, D]])
                v16b = _dims(v3[:, :, 0:D],
                             [v3[:].ap[0], [192, NCHUNK], [0, 2], [1, D]])
                rqb = _dims(rq, [rq_all[:].ap[0], [1, NCHUNK],
                                 [EDGE_COLS, 2], [0, D]])
                nc.vector.tensor_tensor(out=b12, in0=v16b, in1=rqb, op=mult)
                ps = pp.tile([P, 192], F32, space="PSUM", tag="ps")
                for j in range(NCHUNK):
                    nc.tensor.matmul(
                        out=ps[:], lhsT=s_t[:, j, :], rhs=v3[:, j, :],
                        start=(j == 0), stop=(j == NCHUNK - 1))
                y1_t = evp.tile([P, 192], F32, tag="y1")
                nc.scalar.activation(y1_t[:], ps[:], COPY,
                                     scale=degrev_all[:, t:t + 1])
                nc.sync.dma_start(out=y1_bounce[t * P:(t + 1) * P, :], in_=y1_t[:])
                # xs0 blocks 1-2 = x * {rsqrt(deg_row), sqrt(deg_row)}
                x_t = x0p.tile([P, D], F32, tag="xt")
                nc.sync.dma_start(out=x_t[:], in_=x_sh[t * P:(t + 1) * P, :])
                xs0_t = x0p.tile([P, 2, D], F32, tag="xs0")
                xb = _dims(x_t[:], [x_t[:].ap[0], [0, 2], [1, D]])
                rq0b = _dims(rq0_all[:, :, t:t + 1],
                             [rq0_all[:].ap[0], [NT, 2], [0, D]])
                nc.vector.tensor_tensor(out=xs0_t[:], in0=xb, in1=rq0b, op=mult)
                x12dst = _dims(xs0_l[t * P:(t + 1) * P, D:3 * D],
                               [[192, P], [1, 2 * D]])
                nc.sync.dma_start(out=x12dst, in_=xs0_t[:])

            # ================= AllGather =================
            if "ag" not in ab:
                nc.gpsimd.collective_compute(
                "AllGather", mybir.AluOpType.bypass,
                    replica_groups=[list(range(NC))],
                    ins=[y1_bounce[:]], outs=[y1full[:]],
                )

            # ================= hop 2 =================
            for t in range(NT if "hop2" not in ab else 0):
                v2 = v2p.tile([P, NCHUNK, 192], F32, tag="v2")
                if t < 3 or "gather2" in ab:
                    nc.gpsimd.memset(v2[:], 0.0)
                for j in range(NCHUNK if "gather2" not in ab else 0):
                    col = t * NCHUNK + j
                    nc.gpsimd.indirect_dma_start(
                        out=v2[:, j, :], out_offset=None, in_=y1full[:],
                        in_offset=bass.IndirectOffsetOnAxis(
                            ap=idx1_t[:, col:col + 1], axis=0),
                        bounds_check=bnd_pad, oob_is_err=False,
                    )
                s_t = build_s(t)
                v216 = v3p.tile([P, NCHUNK, 192], F16, tag="v216")
                nc.scalar.activation(v216[:], v2[:], COPY)
                ps = pp.tile([P, 192], F32, space="PSUM", tag="ps")
                for j in range(NCHUNK):
                    nc.tensor.matmul(
                        out=ps[:], lhsT=s_t[:, j, :], rhs=v216[:, j, :],
                        start=(j == 0), stop=(j == NCHUNK - 1))
                tmp = evp.tile([P, 192], F32, tag="tmp2")
                nc.scalar.activation(tmp[:], ps[:], COPY,
                                     scale=degrev_all[:, t:t + 1])
                xs0_t = x0p.tile([P, 192], F32, tag="xs0r")
                nc.sync.dma_start(out=xs0_t[:], in_=xs0_l[t * P:(t + 1) * P, :])
                xs2_t = evp.tile([P, 192], F32, tag="xs2")
                nc.vector.tensor_tensor(out=xs2_t[:], in0=tmp[:], in1=xs0_t[:], op=sub)
                nc.sync.dma_start(out=xs2_l[t * P:(t + 1) * P, :], in_=xs2_t[:])

            # ================= final gather + transpose =================
            tables = [xs0_l, y1full, xs2_l]
            fidx = [fidx_loc_t, fidx_mid_t, fidx_loc_t]
            bounds = [bnd_sh, bnd_pad, bnd_sh]
            for pc_i in range(FCH // PC if "final" not in ab else 0):
                gs = []
                for h in range(3):
                    g = gp.tile([P, PC, 192], F32, tag=f"g{h}")
                    if pc_i < 2 or "gatherf" in ab:
                        nc.gpsimd.memset(g[:], 0.0)
                    for j in range(PC if "gatherf" not in ab else 0):
                        col = pc_i * PC + j
                        nc.gpsimd.indirect_dma_start(
                            out=g[:, j, :], out_offset=None, in_=tables[h][:],
                            in_offset=bass.IndirectOffsetOnAxis(
                                ap=fidx[h][:, col:col + 1], axis=0),
                            bounds_check=bounds[h], oob_is_err=False,
                        )
                    gs.append(g)
                stage = stp.tile([P, PC, D * 9], F32, tag="stage")
                for k in range(9):
                    h, b = divmod(k, 3)
                    src = gs[h][:, :, b * D:(b + 1) * D]
                    dst = _dims(stage[:, :, k:k + 1],
                                [stage[:].ap[0], [D * 9, PC], [9, D]])
                    if k % 2 == 0:
                        nc.vector.tensor_copy(out=dst, in_=src)
                    else:
                        nc.scalar.activation(dst, src, COPY)
                obase = out_f[pc_i * PC * P:(pc_i + 1) * PC * P, :]
                orows = _dims(obase, [[576, P], [P * 576, PC], [1, 576]])
                nc.sync.dma_start(out=orows, in_=stage[:])

    _split_multi_waits(nc)
    return nc


def _plan(x, deg, adj_row, adj_col, edge):
    """Host-side sharding: pure index bookkeeping + input reordering."""
    x = np.asarray(x, np.float32)
    deg = np.asarray(deg, np.float32).reshape(-1)
    adj_row = np.asarray(adj_row, np.int64)
    adj_col = np.asarray(adj_col, np.int64)
    edge = np.asarray(edge, np.int64)

    x_full = np.zeros((NPAD, D), np.float32)
    x_full[:N] = x
    iota_np = np.tile(np.arange(P, dtype=np.float16), (P, 1))
    ep = edge.reshape(-1)

    in_maps, positions = [], []
    for c in range(NC):
        r0 = c * NSH
        idx1 = np.full((P, EDGE_COLS), BIG, np.int32)
        rowloc = np.full((P, EDGE_COLS), -1.0, np.float16)
        degcol = np.ones((P, EDGE_COLS), np.float32)
        for t in range(NT):
            base = r0 + t * P
            lo = np.searchsorted(adj_row, base, side="left")
            hi = np.searchsorted(adj_row, base + P, side="left")
            n_e = hi - lo
            assert n_e <= NCHUNK * P, f"tile overflow: {n_e}"
            sl = np.arange(n_e)
            jj, pp_ = divmod(sl, P)
            colbase = t * NCHUNK
            idx1[pp_, colbase + jj] = adj_col[lo:hi]
            rowloc[pp_, colbase + jj] = (adj_row[lo:hi] - base).astype(np.float16)
            degcol[pp_, colbase + jj] = deg[adj_col[lo:hi]]
        real = min(NSH, max(0, N - r0))
        dlocal = np.ones(NSH, np.float32)
        dlocal[:real] = deg[r0:r0 + real]
        degsh = dlocal.reshape(NT, P).T.copy()

        x_shard = np.zeros((NSH, D), np.float32)
        x_shard[:real] = x[r0:r0 + real]

        mine = np.nonzero((ep >= r0) & (ep < r0 + NSH))[0]
        n_c = len(mine)
        assert n_c <= FCH * P, f"endpoint overflow: {n_c}"
        fidx_loc = np.full((P, FCH), BIG, np.int32)
        fidx_mid = np.full((P, FCH), BIG, np.int32)
        sl = np.arange(n_c)
        jj, pp_ = divmod(sl, P)
        fidx_loc[pp_, jj] = (ep[mine] - r0).astype(np.int32)
        fidx_mid[pp_, jj] = ep[mine].astype(np.int32)
        positions.append(mine)

        in_maps.append({
            "x_full": x_full,
            "x_sh": x_shard,
            "degsh": degsh,
            "idx1": idx1,
            "rowloc": rowloc,
            "degcol": degcol,
            "fidx_loc": fidx_loc,
            "fidx_mid": fidx_mid,
            "iota": iota_np,
        })
    return in_maps, positions


def _assemble(results, positions):
    out = np.zeros((2 * EQ, 576), np.float32)
    for c in range(NC):
        rows = results[c]["out_f"]
        n_c = len(positions[c])
        out[positions[c]] = rows[:n_c]
    return out.reshape(2, EQ, D, 9)


def kernel(x, deg, adj_row, adj_col, edge):
    import time
    if "nc" not in _prog_cache:
        t0 = time.time()
        _prog_cache["nc"] = _build_program()
        print(f"[kernel] program build: {time.time()-t0:.1f}s", flush=True)
    nc = _prog_cache["nc"]
    t0 = time.time()
    in_maps, positions = _plan(x, deg, adj_row, adj_col, edge)
    print(f"[kernel] host plan: {time.time()-t0:.1f}s", flush=True)
    t0 = time.time()
    res = run_bass_kernel_spmd(nc, in_maps, list(range(NC)))
    print(f"[kernel] compile+run: {time.time()-t0:.1f}s", flush=True)
    return _assemble(res.results, positions)

